# revision 23
# baseline (speedup 1.0000x reference)
"""Trainium2 Bass kernel for CrossModalAttention (v2).

Reference computation (per (b, m) of B=4 x M=3):
    Q = x_q @ Wq.T + bq ; K = x_k @ Wk.T + bk ; V = x_v @ Wv.T + bv
    per head h (4 heads of dim 128):
        scores = Q_h @ K_h.T / sqrt(128)      [2048, 2048]
        attn   = softmax(scores, axis=-1)
        out_h  = attn @ V_h                   [2048, 128]

Sharding over 8 cores: 48 (b*m, head) units, 6 per core.
  core c: slot A = bm c      (all 4 heads)
          slot B = bm 8+c//2 (heads {0,1} if c even else {2,3})

On-device strategy per slot (v2):
  - scores stay transposed (ST[k, q] = K @ Q.T) so softmax k-sums and the
    attn@V contraction keep k on partitions; no max-subtraction (scores are
    O(0.2), exp cannot overflow, softmax is shift-invariant)
  - Q,K are quantized to fp8e4 at the projection bias-add and shuffled into
    a [64, 2, tok] d-pair layout (2 small SBUF DMAs per head) so the scores
    matmul runs in DoubleRow perf mode at 2x rate; error impact ~1% via
    softmax, well under the 2e-2 budget
  - the softmax denominator is a PE matmul chain with an all-ones
    [128, 128] stationary against the E tiles, producing den[q] replicated
    across all psum partitions (partition-broadcast for free) - the PE
    streams E at 315G elem/s, far faster than any tree-sum on DVE/gpsimd,
    and no PE transposes or partition-axis reductions are needed
  - attn@V keeps V stationary (out[d, q] psum; ldweights hide under the
    512-col moving side); the output is written TRANSPOSED to HBM
    ([D, NTOK]) and the host transposes back, so no PE output transposes
  - softmax division: reciprocal of the replicated den, one tensor_tensor
    multiply, then the V bias folds into a tensor_scalar_add with bv as a
    per-partition (per-d) scalar; out = attn@V_nobias / den + bv since
    sum(attn) == 1
  - emission is software-pipelined: per iteration the PE runs
    den(u-1), scores(u), attn@V(u-1) so exp latency and the
    reciprocal->broadcast round trip hide behind matmuls; slot B's
    projections are emitted piecewise between slot A attention units to
    keep the scalar engine fed while the PE projects
"""

import sys
import os

for _p in ("/root/.axon_site/_ro/trn_rl_repo", "/opt/trn_rl_repo"):
    if os.path.isdir(_p) and _p not in sys.path:
        sys.path.append(_p)

import numpy as np
import ml_dtypes

import concourse.bass as bass
import concourse.tile as tile
from concourse import bacc, mybir
from concourse.bass_utils import run_bass_kernel_spmd

B, M, NTOK, DIM = 4, 3, 2048, 512
H, HD = 4, 128
NBM = B * M  # 12
NCORES = 8
SCALE = 1.0 / float(np.sqrt(HD))

F32 = mybir.dt.float32
BF16 = mybir.dt.bfloat16
FP8 = mybir.dt.float8e4

TT = NTOK // 128  # 16 token tiles
CT = DIM // 128  # 4 contraction tiles
QCH = 512  # q is processed in chunks of 512
NQC = NTOK // QCH  # 4
NSUB = QCH // 128  # 4 q sub-chunks per chunk

SCORES_FP8 = True

# Knobs the test harness may flip before calling kernel():
TRACE = False
TRACE_KWARGS = {}
LAST_RESULTS = None

MULT = mybir.AluOpType.mult
ADD = mybir.AluOpType.add
DR = mybir.MatmulPerfMode.DoubleRow


class Slot:
    def __init__(self, name, nh):
        self.name = name
        self.nh = nh
        self.D = nh * HD


def _emit_proj(nc, pools, dram, sl):
    """Generator emitting projection pieces for one slot; yields between
    pieces so the caller can interleave them with attention units."""
    s, nh, D = sl.name, sl.nh, sl.D
    (xtp, q8p, qt8p, vp, wp, ep, recp, outp, biasp, ppv, pst, pden, pout) = pools

    # ---- weights + biases (small; one piece) ----
    ws = {}
    for wname in ("wq", "wk", "wv"):
        w = wp.tile([128, CT, D], BF16, tag=f"{wname}_{s}")
        nc.sync.dma_start(
            out=w[:, :, :],
            in_=dram[f"{wname}_{s}"][:].rearrange("(c p) d -> p c d", p=128),
        )
        ws[wname] = w
    bqk = biasp.tile([128, 2, nh], F32, tag=f"bqk_{s}")
    nc.sync.dma_start(
        out=bqk[:, 0, :], in_=dram[f"bq_{s}"][:].rearrange("(j p) -> p j", p=128)
    )
    nc.sync.dma_start(
        out=bqk[:, 1, :], in_=dram[f"bk_{s}"][:].rearrange("(j p) -> p j", p=128)
    )
    # bv as a per-partition (per-d) column per head: bvc[p, h] = bv[h*128+p]
    bvc = biasp.tile([128, nh], F32, tag=f"bvc_{s}")
    nc.sync.dma_start(
        out=bvc[:, :], in_=dram[f"bv_{s}"][:].rearrange("(h p) -> p h", p=128)
    )
    sl.bvc = bvc

    def load_xt(xname):
        # per-ct tiles so each consumer matmul waits only on its own slice
        xr = dram[f"{xname}_{s}"][:].rearrange("M (c p) -> M c p", p=128)
        xts = []
        for ct in range(CT):
            xt = xtp.tile([128, NTOK], BF16, tag=f"xt{ct}")
            nc.sync.dma_start(out=xt[:, :], in_=xr[:, ct], transpose=True)
            xts.append(xt)
        return xts

    if SCORES_FP8:
        # QT8/KT8 in DoubleRow d-pair layout: [p<64, i<2, h, t] = proj[i*64+p, h, t]
        QT = qt8p.tile([64, 2, nh, NTOK], FP8, tag=f"qt8_{s}")
        KT = qt8p.tile([64, 2, nh, NTOK], FP8, tag=f"kt8_{s}")
    else:
        QT = qt8p.tile([128, nh, NTOK], BF16, tag=f"qt8_{s}")
        KT = qt8p.tile([128, nh, NTOK], BF16, tag=f"kt8_{s}")
    sl.QT, sl.KT = QT, KT

    for which, (xname, wname, dst) in enumerate((("xq", "wq", QT), ("xk", "wk", KT))):
        xts = load_xt(xname)
        w = ws[wname]
        for dt in range(nh):
            if SCORES_FP8:
                stage = q8p.tile([128, NTOK], FP8, tag="q8")
            for qc in range(NQC):
                ps = ppv.tile([128, QCH], F32, tag="pp")
                for ct in range(CT):
                    nc.tensor.matmul(
                        ps[:, :],
                        w[:, ct, dt * 128 : (dt + 1) * 128],
                        xts[ct][:, qc * QCH : (qc + 1) * QCH],
                        start=(ct == 0),
                        stop=(ct == CT - 1),
                    )
                out_ap = (
                    stage[:, qc * QCH : (qc + 1) * QCH]
                    if SCORES_FP8
                    else dst[:, dt, qc * QCH : (qc + 1) * QCH]
                )
                nc.vector.tensor_scalar_add(out_ap, ps[:, :], bqk[:, which, dt : dt + 1])
            if SCORES_FP8:
                # partition shuffle into the DoubleRow pair layout
                nc.sync.dma_start(out=dst[0:64, 0, dt, :], in_=stage[0:64, :])
                nc.sync.dma_start(out=dst[0:64, 1, dt, :], in_=stage[64:128, :])
            yield

    V = vp.tile([128, TT, D], BF16, tag=f"v_{s}")
    sl.V = V
    xts = load_xt("xv")
    w = ws["wv"]
    for tg in range(4):
        for tt in range(tg * 4, tg * 4 + 4):
            ps = ppv.tile([128, D], F32, tag="pp")
            for ct in range(CT):
                nc.tensor.matmul(
                    ps[:, :],
                    xts[ct][:, tt * 128 : (tt + 1) * 128],
                    w[:, ct, :],
                    start=(ct == 0),
                    stop=(ct == CT - 1),
                )
            nc.vector.tensor_copy(V[:, tt, :], ps[:, :])
        yield


def _emit_scores(nc, pools, sl, h, qc):
    """Scores + exp for one (slot, head, q-chunk) unit; returns the E tile."""
    (xtp, q8p, qt8p, vp, wp, ep, recp, outp, biasp, ppv, pst, pden, pout) = pools
    qsl = slice(qc * QCH, (qc + 1) * QCH)
    E = ep.tile([128, TT, QCH], BF16, tag="E")
    for g in range(TT // 2):
        st = pst.tile([128, 2, QCH], F32, tag="st")
        for j in range(2):
            kt = 2 * g + j
            if SCORES_FP8:
                nc.tensor.matmul(
                    st[:, j, :],
                    sl.KT[:, :, h, kt * 128 : (kt + 1) * 128],
                    sl.QT[:, :, h, qsl],
                    start=True,
                    stop=True,
                    perf_mode=DR,
                )
            else:
                nc.tensor.matmul(
                    st[:, j, :],
                    sl.KT[:, h, kt * 128 : (kt + 1) * 128],
                    sl.QT[:, h, qsl],
                    start=True,
                    stop=True,
                )
        nc.scalar.activation(
            E[:, 2 * g : 2 * g + 2, :],
            st[:, :, :],
            mybir.ActivationFunctionType.Exp,
            scale=SCALE,
        )
    return E


def _emit_den(nc, pools, dram, uidx, sl, h, qc, E, ones):
    """Softmax denominator: an all-ones [128, 128] stationary against E's
    k-tiles gives den[q] replicated across every psum partition - the
    partition-broadcast comes free from the matmul (cost is set by the
    moving side only), so the reciprocal is immediately usable as a
    [128, QCH] tensor operand."""
    (xtp, q8p, qt8p, vp, wp, ep, recp, outp, biasp, ppv, pst, pden, pout) = pools
    pd = pden.tile([128, QCH], F32, tag="pd")
    for kt in range(TT):
        nc.tensor.matmul(
            pd[:, :],
            ones[:, :],
            E[:, kt, :],
            start=(kt == 0),
            stop=(kt == TT - 1),
        )
    recb = recp.tile([128, QCH], F32, tag="recb")
    nc.vector.reciprocal(recb[:, :], pd[:, :])
    return recb


def _emit_attnv(nc, pools, dram, sl, h, qc, E, recb):
    """attn@V (V stationary, out[d, q]) + divide + bias + transposed DMA."""
    (xtp, q8p, qt8p, vp, wp, ep, recp, outp, biasp, ppv, pst, pden, pout) = pools
    out_d = dram[f"out_{sl.name}"]
    pv = pout.tile([128, QCH], F32, tag="pv")
    for kt in range(TT):
        nc.tensor.matmul(
            pv[:, :],
            sl.V[:, kt, h * HD : (h + 1) * HD],
            E[:, kt, :],
            start=(kt == 0),
            stop=(kt == TT - 1),
        )
    tmp = outp.tile([128, QCH], F32, tag="tmp")
    nc.vector.tensor_mul(tmp[:, :], pv[:, :], recb[:, :])
    ot = outp.tile([128, QCH], F32, tag="ot")
    nc.vector.tensor_scalar_add(ot[:, :], tmp[:, :], sl.bvc[:, h : h + 1])
    # output stays transposed in HBM ([D, NTOK]); the host transposes back
    nc.sync.dma_start(
        out=out_d[h * HD : (h + 1) * HD, qc * QCH : (qc + 1) * QCH],
        in_=ot[:, :],
    )


def _build_program():
    # Bacc (not plain Bass): its compile() pipeline legalizes multi-wait
    # instructions (walrus accepts at most 1 sync wait per instruction).
    nc = bacc.Bacc()
    dram = {}
    for s in ("a", "b"):
        D = 512 if s == "a" else 256
        for nm in ("xq", "xk", "xv"):
            dram[f"{nm}_{s}"] = nc.dram_tensor(
                f"{nm}_{s}", [NTOK, DIM], BF16, kind="ExternalInput"
            )
        for nm in ("wq", "wk", "wv"):
            dram[f"{nm}_{s}"] = nc.dram_tensor(
                f"{nm}_{s}", [DIM, D], BF16, kind="ExternalInput"
            )
        for nm in ("bq", "bk", "bv"):
            dram[f"{nm}_{s}"] = nc.dram_tensor(
                f"{nm}_{s}", [D], F32, kind="ExternalInput"
            )
        dram[f"out_{s}"] = nc.dram_tensor(
            f"out_{s}", [D, NTOK], F32, kind="ExternalOutput"
        )


    slot_a = Slot("a", 4)
    slot_b = Slot("b", 2)

    with tile.TileContext(nc) as tc:
        with (
            tc.tile_pool(name="xtp", bufs=2) as xtp,
            tc.tile_pool(name="q8p", bufs=3) as q8p,
            tc.tile_pool(name="qt8p", bufs=1) as qt8p,
            tc.tile_pool(name="vp", bufs=1) as vp,
            tc.tile_pool(name="wp", bufs=1) as wp,
            tc.tile_pool(name="ep", bufs=3) as ep,
            tc.tile_pool(name="recp", bufs=2) as recp,
            tc.tile_pool(name="outp", bufs=2) as outp,
            tc.tile_pool(name="biasp", bufs=1) as biasp,
            tc.tile_pool(name="singles", bufs=1) as singles,
            tc.tile_pool(name="ppv", bufs=2, space="PSUM") as ppv,
            tc.tile_pool(name="pst", bufs=2, space="PSUM") as pst,
            tc.tile_pool(name="pden", bufs=1, space="PSUM") as pden,
            tc.tile_pool(name="pout", bufs=1, space="PSUM") as pout,
        ):
            pools = (xtp, q8p, qt8p, vp, wp, ep, recp, outp, biasp, ppv, pst, pden, pout)

            ones = singles.tile([128, 128], BF16, tag="ones")
            nc.vector.memset(ones[:, :], 1.0)

            units = [(slot_a, h, qc) for h in range(4) for qc in range(NQC)] + [
                (slot_b, h, qc) for h in range(2) for qc in range(NQC)
            ]

            # slot A projections run up front
            for _ in _emit_proj(nc, pools, dram, slot_a):
                pass
            projb = _emit_proj(nc, pools, dram, slot_b)

            # attention pipeline, PE order per iteration:
            #   den(u-1) | scores(u) | attn@V(u-2)
            # attn@V lags two units so the reciprocal -> DRAM-bounce ->
            # broadcast round trip never stalls the PE; slot B projection
            # pieces drain in between
            inflight = []  # [(sl, h, qc, E, recb)]
            for u, (sl, h, qc) in enumerate(units):
                if inflight and inflight[-1][4] is None:
                    psl, ph, pqc, pE, _ = inflight[-1]
                    recb = _emit_den(nc, pools, dram, u - 1, psl, ph, pqc, pE, ones)
                    inflight[-1] = (psl, ph, pqc, pE, recb)
                E = _emit_scores(nc, pools, sl, h, qc)
                if len(inflight) >= 2:
                    _emit_attnv(nc, pools, dram, *inflight.pop(0))
                if u >= 4 and projb is not None:
                    try:
                        next(projb)
                    except StopIteration:
                        projb = None
                inflight.append((sl, h, qc, E, None))
            for i, entry in enumerate(inflight):
                if entry[4] is None:
                    psl, ph, pqc, pE, _ = entry
                    recb = _emit_den(
                        nc, pools, dram, len(units) - len(inflight) + i,
                        psl, ph, pqc, pE, ones,
                    )
                    entry = (psl, ph, pqc, pE, recb)
                _emit_attnv(nc, pools, dram, *entry)

    # Run Bacc's compile pipeline (register allocation, sync-wait
    # legalization, nop fusion) — run_bass_via_pjrt does not call it.
    nc.finalize()
    return nc


_PROGRAM = None


def _get_program():
    global _PROGRAM
    if _PROGRAM is None:
        _PROGRAM = _build_program()
    return _PROGRAM


def kernel(query, key, value, Wq, bq, Wk, bk, Wv, bv):
    global LAST_RESULTS
    bf = ml_dtypes.bfloat16
    q = np.ascontiguousarray(np.asarray(query, np.float32).reshape(NBM, NTOK, DIM)).astype(bf)
    k = np.ascontiguousarray(np.asarray(key, np.float32).reshape(NBM, NTOK, DIM)).astype(bf)
    v = np.ascontiguousarray(np.asarray(value, np.float32).reshape(NBM, NTOK, DIM)).astype(bf)
    WqT = np.ascontiguousarray(np.asarray(Wq, np.float32).T).astype(bf)
    WkT = np.ascontiguousarray(np.asarray(Wk, np.float32).T).astype(bf)
    WvT = np.ascontiguousarray(np.asarray(Wv, np.float32).T).astype(bf)
    bq = np.asarray(bq, np.float32)
    bk = np.asarray(bk, np.float32)
    bv = np.asarray(bv, np.float32)

    in_maps = []
    for c in range(NCORES):
        bm_a = c
        bm_b = 8 + c // 2
        hs = (c % 2) * 256  # head-pair column offset for slot B
        in_maps.append(
            {
                "xq_a": q[bm_a], "xk_a": k[bm_a], "xv_a": v[bm_a],
                "xq_b": q[bm_b], "xk_b": k[bm_b], "xv_b": v[bm_b],
                "wq_a": WqT, "wk_a": WkT, "wv_a": WvT,
                "bq_a": bq, "bk_a": bk, "bv_a": bv,
                "wq_b": np.ascontiguousarray(WqT[:, hs : hs + 256]),
                "wk_b": np.ascontiguousarray(WkT[:, hs : hs + 256]),
                "wv_b": np.ascontiguousarray(WvT[:, hs : hs + 256]),
                "bq_b": np.ascontiguousarray(bq[hs : hs + 256]),
                "bk_b": np.ascontiguousarray(bk[hs : hs + 256]),
                "bv_b": np.ascontiguousarray(bv[hs : hs + 256]),
            }
        )

    nc = _get_program()
    res = run_bass_kernel_spmd(
        nc, in_maps, list(range(NCORES)), trace=TRACE, **TRACE_KWARGS
    )
    LAST_RESULTS = res

    out = np.empty((NBM, NTOK, DIM), np.float32)
    for c in range(NCORES):
        hs = (c % 2) * 256
        out[c] = res.results[c]["out_a"].T
        out[8 + c // 2][:, hs : hs + 256] = res.results[c]["out_b"].T
    return out.reshape(B, M, NTOK, DIM)
